# revision 3
# baseline (speedup 1.0000x reference)
"""Trainium2 Bass kernel for nn_Decoder (matmul + BatchNorm + MultiStepLIF).

Reference computation (TB=64, N=1024, C=768, T=4):
  y = x @ W.T                       (TB, N, C)
  y -> (TB, C, N); BatchNorm over channel axis with batch stats
  reshape (T, B, C, N) -> swap -> (T, B, N, C); LIF scan over T
  out = swap(reshape(spikes, (TB, C, N)))   -- a genuine (n,c)->(c',n')
        flat-index remap, NOT an inverse transpose.

Strategy (8 cores, data-parallel over batch B):
  - core s handles b in {2s, 2s+1}: 8 (t,b) slices of x.
  - matmul as y^T[o, n] = W^T.T @ x^T tiles so BN channel axis lands on
    partitions.  x tiles are PE-transposed on device; both x^T and W are
    split hi/lo bf16 and multiplied in 3 passes (hh + lh + hl) with fp32
    PSUM accumulation (rel err ~1e-5 vs fp32).
  - BN stats fused into PSUM eviction (ACT accum_out for sum, DVE
    scalar_tensor_tensor for sumsq), AllReduced across 8 cores (6 KB).
  - LIF unrolled over T with fused scalar_tensor_tensor ops; the spike
    threshold is fused into the output permutation as strided stripe
    writes (is_ge) into staging tiles, which DMA out contiguously.

The output permutation math: element (n, c) of slice tb has flat index
j = n*C + c and lands at out[tb, j % N, j // N].  With n = 4*n3 + n2,
c = 128*cb + p:  code = 6*n2 + cb in [0, 24) maps bijectively to
(pt, r) = (code % 8, code // 8); dest = stage[pt][p, r + 3*n3] reading
src v[cb][p, n2 + 4*n3].  Partition index p is preserved.
"""

import os

import numpy as np

import concourse.bacc as bacc
import concourse.bass as bass
import concourse.mybir as mybir
import concourse.tile as tile
from concourse import bass_utils

F32 = mybir.dt.float32
BF16 = mybir.dt.bfloat16
ALU = mybir.AluOpType
ACTF = mybir.ActivationFunctionType

N_CORES = 8
T, B, N, C = 4, 16, 1024, 768
B_SH = B // N_CORES          # batches per core
SL = T * B_SH                # (t, b) slices per core
KT = C // 128                # contraction tiles
OT = C // 128                # output-channel tiles
NB = N // 128                # n 128-blocks
NH = N // 512                # n 512-halves
COUNT = float(T * B * N)     # BN reduction count
EPS = 1e-5

_CACHE: dict = {}


def build(num_devices=N_CORES, skip_collective=False):
    nc = bacc.Bacc(
        "TRN2", target_bir_lowering=False, debug=False, num_devices=num_devices
    )
    xs = nc.dram_tensor("xs", [SL, N, C], F32, kind="ExternalInput")
    wth = nc.dram_tensor("wth", [C, C], BF16, kind="ExternalInput")   # W^T hi
    wtl = nc.dram_tensor("wtl", [C, C], BF16, kind="ExternalInput")   # W^T lo
    gm = nc.dram_tensor("gm", [C], F32, kind="ExternalInput")
    bt = nc.dram_tensor("bt", [C], F32, kind="ExternalInput")
    idm = nc.dram_tensor("idm", [128, 128], F32, kind="ExternalInput")
    out = nc.dram_tensor("out", [SL, N, C], F32, kind="ExternalOutput")

    with tile.TileContext(nc) as tc:
        with (
            tc.tile_pool(name="wp", bufs=1) as wp,
            tc.tile_pool(name="xp", bufs=10) as xp,
            tc.tile_pool(name="ap", bufs=3) as ap_,
            tc.tile_pool(name="yp", bufs=4) as yp,
            tc.tile_pool(name="sp", bufs=1) as sp,
            tc.tile_pool(name="bp", bufs=4) as bp,
            tc.tile_pool(name="vp", bufs=3) as vp,
            tc.tile_pool(name="sg", bufs=1) as sg,
            tc.tile_pool(name="tp", bufs=3, space="PSUM") as tpp,
            tc.tile_pool(name="mp", bufs=4, space="PSUM") as mpp,
            tc.tile_pool(name="dr", bufs=1, space="DRAM") as dr,
        ):
            # ---- constants / weights -------------------------------------
            w_hi = wp.tile([128, KT, C], BF16)
            w_lo = wp.tile([128, KT, C], BF16)
            nc.sync.dma_start(
                w_hi[:], wth.ap().rearrange("(kt k) o -> k kt o", k=128)
            )
            nc.sync.dma_start(
                w_lo[:], wtl.ap().rearrange("(kt k) o -> k kt o", k=128)
            )
            id_sb = wp.tile([128, 128], F32)
            nc.sync.dma_start(id_sb[:], idm.ap())
            g_sb = sp.tile([128, OT], F32)
            nc.sync.dma_start(g_sb[:], gm.ap().rearrange("(ot p) -> p ot", p=128))
            b_sb = sp.tile([128, OT], F32)
            nc.sync.dma_start(b_sb[:], bt.ap().rearrange("(ot p) -> p ot", p=128))

            y_dram = dr.tile([SL, OT, 128, N], F32)

            sums = [
                sp.tile([128, SL * NH], F32, tag=f"sum{o}", name=f"sums{o}")
                for o in range(OT)
            ]
            sqs = [
                sp.tile([128, SL * NH], F32, tag=f"sq{o}", name=f"sqs{o}")
                for o in range(OT)
            ]

            # ---- pass A: transpose+split, matmul, stats, y -> DRAM -------
            for i in range(SL):
                xt = []
                for nb in range(NB):
                    t_ = xp.tile([128, C], F32, tag="x")
                    nc.sync.dma_start(t_[:], xs.ap()[i, nb * 128 : (nb + 1) * 128, :])
                    xt.append(t_)
                for nh in range(NH):
                    at_hi = ap_.tile([128, KT, 512], BF16, tag="ahi")
                    at_lo = ap_.tile([128, KT, 512], BF16, tag="alo")
                    for kt in range(KT):
                        ps = tpp.tile([128, 512], F32, tag="tp")
                        for q in range(4):
                            nb = nh * 4 + q
                            nc.tensor.transpose(
                                ps[:, q * 128 : (q + 1) * 128],
                                xt[nb][:, kt * 128 : (kt + 1) * 128],
                                id_sb[:],
                            )
                        hi_ap = at_hi[:, kt, :]
                        lo_ap = at_lo[:, kt, :]
                        nc.scalar.copy(hi_ap, ps[:])  # f32 -> bf16 cast
                        nc.vector.scalar_tensor_tensor(
                            lo_ap, ps[:], 1.0, hi_ap, op0=ALU.mult, op1=ALU.subtract
                        )
                    for ot in range(OT):
                        psm = mpp.tile([128, 512], F32, tag="mm")
                        passes = [(w_hi, at_hi), (w_lo, at_hi), (w_hi, at_lo)]
                        for kt in range(KT):
                            for pi, (wop, aop) in enumerate(passes):
                                nc.tensor.matmul(
                                    psm[:],
                                    wop[:, kt, ot * 128 : (ot + 1) * 128],
                                    aop[:, kt, :],
                                    start=(kt == 0 and pi == 0),
                                    stop=(kt == KT - 1 and pi == 2),
                                )
                        ysb = yp.tile([128, 512], F32, tag="y")
                        idx = i * NH + nh
                        nc.scalar.activation(
                            ysb[:],
                            psm[:],
                            ACTF.Copy,
                            accum_out=sums[ot][:, idx : idx + 1],
                        )
                        sq = yp.tile([128, 512], F32, tag="sq")
                        nc.vector.scalar_tensor_tensor(
                            sq[:],
                            ysb[:],
                            1.0,
                            ysb[:],
                            op0=ALU.mult,
                            op1=ALU.mult,
                            accum_out=sqs[ot][:, idx : idx + 1],
                        )
                        nc.sync.dma_start(
                            y_dram[i, ot, :, nh * 512 : (nh + 1) * 512], ysb[:]
                        )

            # ---- stats reduce + AllReduce + per-channel affine -----------
            stat = sp.tile([128, 2 * OT], F32)
            for ot in range(OT):
                nc.vector.tensor_reduce(
                    stat[:, ot : ot + 1], sums[ot][:], axis=mybir.AxisListType.X,
                    op=ALU.add,
                )
                nc.vector.tensor_reduce(
                    stat[:, OT + ot : OT + ot + 1], sqs[ot][:],
                    axis=mybir.AxisListType.X, op=ALU.add,
                )
            ar_in = dr.tile([128, 2 * OT], F32)
            ar_out = dr.tile([128, 2 * OT], F32, addr_space="Shared")
            nc.sync.dma_start(ar_in[:], stat[:])
            if skip_collective:
                nc.gpsimd.dma_start(ar_out[:], ar_in[:])
            else:
                nc.gpsimd.collective_compute(
                    "AllReduce",
                    ALU.add,
                    replica_groups=[list(range(num_devices))],
                    ins=[ar_in.opt()],
                    outs=[ar_out.opt()],
                )
            ars = sp.tile([128, 2 * OT], F32)
            nc.sync.dma_start(ars[:], ar_out[:])

            meanv = sp.tile([128, OT], F32)
            nc.vector.tensor_scalar_mul(meanv[:], ars[:, 0:OT], 1.0 / COUNT)
            e2 = sp.tile([128, OT], F32)
            nc.vector.tensor_scalar_mul(e2[:], ars[:, OT : 2 * OT], 1.0 / COUNT)
            m2 = sp.tile([128, OT], F32)
            nc.vector.scalar_tensor_tensor(
                m2[:], meanv[:], 0.0, meanv[:], op0=ALU.bypass, op1=ALU.mult
            )
            varep = sp.tile([128, OT], F32)
            nc.vector.scalar_tensor_tensor(
                varep[:], m2[:], -1.0, e2[:], op0=ALU.mult, op1=ALU.add
            )
            nc.vector.tensor_scalar_add(varep[:], varep[:], EPS)
            sq_ = sp.tile([128, OT], F32)
            nc.scalar.sqrt(sq_[:], varep[:])
            rstd = sp.tile([128, OT], F32)
            nc.vector.reciprocal(rstd[:], sq_[:])
            a_ = sp.tile([128, OT], F32)
            nc.vector.scalar_tensor_tensor(
                a_[:], g_sb[:], 0.0, rstd[:], op0=ALU.bypass, op1=ALU.mult
            )
            scalew = sp.tile([128, OT], F32)
            nc.vector.tensor_scalar_mul(scalew[:], a_[:], 0.5)
            ma = sp.tile([128, OT], F32)
            nc.vector.scalar_tensor_tensor(
                ma[:], meanv[:], 0.0, a_[:], op0=ALU.bypass, op1=ALU.mult
            )
            bhalf = sp.tile([128, OT], F32)
            nc.vector.tensor_scalar_mul(bhalf[:], b_sb[:], 0.5)
            biasw = sp.tile([128, OT], F32)
            nc.vector.scalar_tensor_tensor(
                biasw[:], ma[:], -0.5, bhalf[:], op0=ALU.mult, op1=ALU.add
            )

            # ---- pass B: normalize, LIF, permuted spike stripes ----------
            for j in range(B_SH):
                stg = {}
                for t_ in range(T):
                    for pt in range(NB):
                        stg[(t_, pt)] = sg.tile(
                            [128, C], BF16, tag=f"stg{t_}_{pt}",
                            name=f"stg{j}_{t_}_{pt}",
                        )
                for cb in range(KT):
                    w_tiles = []
                    for t_ in range(T):
                        i = t_ * B_SH + j
                        yt = bp.tile([128, N], F32, tag="yb")
                        nc.sync.dma_start(yt[:], y_dram[i, cb, :, :])
                        wt_ = bp.tile([128, N], F32, tag="wb")
                        nc.scalar.activation(
                            wt_[:],
                            yt[:],
                            ACTF.Identity,
                            bias=biasw[:, cb : cb + 1],
                            scale=scalew[:, cb : cb + 1],
                        )
                        w_tiles.append(wt_)
                    v_list = [w_tiles[0]]
                    cur = w_tiles[0]
                    for t_ in range(1, T):
                        vr = vp.tile([128, N], F32, tag="v")
                        nc.vector.scalar_tensor_tensor(
                            vr[:], cur[:], 1.0, cur[:], op0=ALU.is_lt, op1=ALU.mult
                        )
                        vn = vp.tile([128, N], F32, tag="v")
                        nc.vector.scalar_tensor_tensor(
                            vn[:], vr[:], 0.5, w_tiles[t_][:],
                            op0=ALU.mult, op1=ALU.add,
                        )
                        v_list.append(vn)
                        cur = vn
                    for t_ in range(T):
                        vv = v_list[t_][:].rearrange("p (n3 n2) -> p n2 n3", n2=4)
                        for n2 in range(4):
                            code = 6 * n2 + cb
                            pt, r = code % 8, code // 8
                            dst = (
                                stg[(t_, pt)][:]
                                .rearrange("p (n3 r) -> p r n3", r=3)[:, r, :]
                            )
                            src = vv[:, n2, :]
                            eng = nc.vector if n2 % 2 == 0 else nc.gpsimd
                            eng.tensor_scalar(
                                dst, src, 1.0, None, op0=ALU.is_ge
                            )
                for t_ in range(T):
                    i = t_ * B_SH + j
                    for pt in range(NB):
                        nc.gpsimd.dma_start(
                            out.ap()[i, pt * 128 : (pt + 1) * 128, :],
                            stg[(t_, pt)][:],
                        )

    nc.compile()
    return nc


def _prep_inputs(x, W, gamma, beta):
    import ml_dtypes

    wt = np.ascontiguousarray(W.T.astype(np.float32))
    wth = wt.astype(ml_dtypes.bfloat16)
    wtl = (wt - wth.astype(np.float32)).astype(ml_dtypes.bfloat16)
    idm = np.eye(128, dtype=np.float32)
    xv = np.asarray(x, dtype=np.float32).reshape(T, B, N, C)
    in_maps = []
    for s in range(N_CORES):
        xs = np.ascontiguousarray(
            xv[:, s * B_SH : (s + 1) * B_SH].reshape(SL, N, C)
        )
        in_maps.append(
            {
                "xs": xs,
                "wth": np.asarray(wth),
                "wtl": np.asarray(wtl),
                "gm": np.asarray(gamma, dtype=np.float32),
                "bt": np.asarray(beta, dtype=np.float32),
                "idm": idm,
            }
        )
    return in_maps


def kernel(x, W, gamma, beta, T=4, **_unused):
    assert int(T) == 4
    if "nc" not in _CACHE:
        _CACHE["nc"] = build()
    nc = _CACHE["nc"]
    in_maps = _prep_inputs(x, W, gamma, beta)
    res = bass_utils.run_bass_kernel_spmd(
        nc, in_maps, core_ids=list(range(N_CORES))
    )
    out = np.empty((T * B // 4 * 4, N, C), dtype=np.float32)  # (64, N, C)
    ov = out.reshape(4, B, N, C)
    for s in range(N_CORES):
        ov[:, s * B_SH : (s + 1) * B_SH] = res.results[s]["out"].reshape(
            4, B_SH, N, C
        )
    return out


if __name__ == "__main__":
    if os.environ.get("TLSIM"):
        from concourse.timeline_sim import TimelineSim

        nc = build(num_devices=1, skip_collective=True)
        ts = TimelineSim(nc, trace=False)
        t = ts.simulate()
        print(f"timeline_sim total = {t / 1e3:.1f} us")
    else:
        rng = np.random.default_rng(0)
        x = rng.standard_normal((T * B, N, C), dtype=np.float32)
        W = (rng.standard_normal((C, C)) / np.sqrt(C)).astype(np.float32)
        gamma = np.ones(C, np.float32)
        beta = np.zeros(C, np.float32)
        o = kernel(x, W, gamma, beta, 4)
        print("ran, out shape", o.shape, "mean spike", o.mean())


# revision 7
# speedup vs baseline: 1.0015x; 1.0015x over previous
"""Trainium2 Bass kernel for nn_Decoder (matmul + BatchNorm + MultiStepLIF).

Reference computation (TB=64, N=1024, C=768, T=4):
  y = x @ W.T                       (TB, N, C)
  y -> (TB, C, N); BatchNorm over channel axis with batch stats
  reshape (T, B, C, N) -> swap -> (T, B, N, C); LIF scan over T
  out = swap(reshape(spikes, (TB, C, N)))   -- a genuine (n,c)->(c',n')
        flat-index remap, NOT an inverse transpose.

Strategy (8 cores, data-parallel over batch B):
  - core s handles b in {2s, 2s+1}: 8 (t,b) slices of x.
  - matmul as y^T[o, n] = W^T.T @ x^T tiles so BN channel axis lands on
    partitions.  x tiles are PE-transposed on device; both x^T and W are
    split hi/lo bf16 and multiplied in 3 passes (hh + lh + hl) with fp32
    PSUM accumulation (rel err ~1e-5 vs fp32).
  - BN stats fused into PSUM eviction (ACT accum_out for sum, DVE
    scalar_tensor_tensor for sumsq), AllReduced across 8 cores (6 KB).
  - LIF unrolled over T with fused scalar_tensor_tensor ops; the spike
    threshold is fused into the output permutation as strided stripe
    writes (is_ge) into staging tiles, which DMA out contiguously.

The output permutation math: element (n, c) of slice tb has flat index
j = n*C + c and lands at out[tb, j % N, j // N].  With n = 4*n3 + n2,
c = 128*cb + p:  code = 6*n2 + cb in [0, 24) maps bijectively to
(pt, r) = (code % 8, code // 8); dest = stage[pt][p, r + 3*n3] reading
src v[cb][p, n2 + 4*n3].  Partition index p is preserved.
"""

import os

import numpy as np

import concourse.bacc as bacc
import concourse.bass as bass
import concourse.mybir as mybir
import concourse.tile as tile
from concourse import bass_utils

F32 = mybir.dt.float32
BF16 = mybir.dt.bfloat16
ALU = mybir.AluOpType
ACTF = mybir.ActivationFunctionType

N_CORES = 8
T, B, N, C = 4, 16, 1024, 768
B_SH = B // N_CORES          # batches per core
SL = T * B_SH                # (t, b) slices per core
KT = C // 128                # contraction tiles
OT = C // 128                # output-channel tiles
NB = N // 128                # n 128-blocks
NH = N // 512                # n 512-halves
COUNT = float(T * B * N)     # BN reduction count
EPS = 1e-5

_CACHE: dict = {}


def build(num_devices=N_CORES, skip_collective=False, host_prep=False,
          skip_b=False, lif_pool_frac=0.0, stripe_dve_of16=1):
    nc = bacc.Bacc(
        "TRN2", target_bir_lowering=False, debug=False, num_devices=num_devices
    )
    if host_prep:
        xth = nc.dram_tensor("xth", [SL, C, N], BF16, kind="ExternalInput")
        xtl = nc.dram_tensor("xtl", [SL, C, N], BF16, kind="ExternalInput")
    else:
        xs = nc.dram_tensor("xs", [SL, N, C], F32, kind="ExternalInput")
    wth = nc.dram_tensor("wth", [C, C], BF16, kind="ExternalInput")   # W^T hi
    wtl = nc.dram_tensor("wtl", [C, C], BF16, kind="ExternalInput")   # W^T lo
    gm = nc.dram_tensor("gm", [C], F32, kind="ExternalInput")
    bt = nc.dram_tensor("bt", [C], F32, kind="ExternalInput")
    idm = None
    if not host_prep:
        idm = nc.dram_tensor("idm", [128, 128], F32, kind="ExternalInput")
    out = nc.dram_tensor("out", [SL, N, C], F32, kind="ExternalOutput")

    with tile.TileContext(nc) as tc:
        with (
            tc.tile_pool(name="wp", bufs=1) as wp,
            tc.tile_pool(name="xp", bufs=10) as xp,
            tc.tile_pool(name="ap", bufs=3) as ap_,
            tc.tile_pool(name="yp", bufs=4) as yp,
            tc.tile_pool(name="sp", bufs=1) as sp,
            tc.tile_pool(name="bp", bufs=5) as bp,
            tc.tile_pool(name="vp", bufs=4) as vp,
            tc.tile_pool(name="sg", bufs=1) as sg,
            tc.tile_pool(name="tp", bufs=3, space="PSUM") as tpp,
            tc.tile_pool(name="mp", bufs=4, space="PSUM") as mpp,
            tc.tile_pool(name="dr", bufs=1, space="DRAM") as dr,
        ):
            # ---- constants / weights -------------------------------------
            w_hi = wp.tile([128, KT, C], BF16)
            w_lo = wp.tile([128, KT, C], BF16)
            nc.sync.dma_start(
                w_hi[:], wth.ap().rearrange("(kt k) o -> k kt o", k=128)
            )
            nc.sync.dma_start(
                w_lo[:], wtl.ap().rearrange("(kt k) o -> k kt o", k=128)
            )
            if not host_prep:
                id_sb = wp.tile([128, 128], F32)
                nc.sync.dma_start(id_sb[:], idm.ap())
            g_sb = sp.tile([128, OT], F32)
            nc.sync.dma_start(g_sb[:], gm.ap().rearrange("(ot p) -> p ot", p=128))
            b_sb = sp.tile([128, OT], F32)
            nc.sync.dma_start(b_sb[:], bt.ap().rearrange("(ot p) -> p ot", p=128))

            y_dram = dr.tile([SL, OT, 128, N], F32)

            sums = [
                sp.tile([128, SL * NH], F32, tag=f"sum{o}", name=f"sums{o}")
                for o in range(OT)
            ]
            sqs = [
                sp.tile([128, SL * NH], F32, tag=f"sq{o}", name=f"sqs{o}")
                for o in range(OT)
            ]

            # ---- pass A: transpose+split, matmul, stats, y -> DRAM -------
            for i in range(SL):
                if not host_prep:
                    xt = []
                    for nb in range(NB):
                        t_ = xp.tile([128, C], F32, tag="x")
                        nc.sync.dma_start(
                            t_[:], xs.ap()[i, nb * 128 : (nb + 1) * 128, :]
                        )
                        xt.append(t_)
                for nh in range(NH):
                    at_hi = ap_.tile([128, KT, 512], BF16, tag="ahi")
                    at_lo = ap_.tile([128, KT, 512], BF16, tag="alo")
                    if host_prep:
                        nc.sync.dma_start(
                            at_hi[:],
                            xth.ap()[i].rearrange("(kt k) n -> k kt n", k=128)[
                                :, :, nh * 512 : (nh + 1) * 512
                            ],
                        )
                        nc.sync.dma_start(
                            at_lo[:],
                            xtl.ap()[i].rearrange("(kt k) n -> k kt n", k=128)[
                                :, :, nh * 512 : (nh + 1) * 512
                            ],
                        )
                    else:
                        for kt in range(KT):
                            ps = tpp.tile([128, 512], F32, tag="tp")
                            for q in range(4):
                                nb = nh * 4 + q
                                nc.tensor.transpose(
                                    ps[:, q * 128 : (q + 1) * 128],
                                    xt[nb][:, kt * 128 : (kt + 1) * 128],
                                    id_sb[:],
                                )
                            hi_ap = at_hi[:, kt, :]
                            lo_ap = at_lo[:, kt, :]
                            nc.scalar.copy(hi_ap, ps[:])  # f32 -> bf16 cast
                            nc.vector.scalar_tensor_tensor(
                                lo_ap, ps[:], 1.0, hi_ap,
                                op0=ALU.mult, op1=ALU.subtract,
                            )
                    for ot in range(OT):
                        psm = mpp.tile([128, 512], F32, tag="mm")
                        passes = [(w_hi, at_hi), (w_lo, at_hi), (w_hi, at_lo)]
                        for kt in range(KT):
                            for pi, (wop, aop) in enumerate(passes):
                                nc.tensor.matmul(
                                    psm[:],
                                    wop[:, kt, ot * 128 : (ot + 1) * 128],
                                    aop[:, kt, :],
                                    start=(kt == 0 and pi == 0),
                                    stop=(kt == KT - 1 and pi == 2),
                                )
                        ysb = yp.tile([128, 512], F32, tag="y")
                        idx = i * NH + nh
                        nc.scalar.activation(
                            ysb[:],
                            psm[:],
                            ACTF.Copy,
                            accum_out=sums[ot][:, idx : idx + 1],
                        )
                        sq = yp.tile([128, 512], F32, tag="sq", bufs=2)
                        nc.scalar.activation(
                            sq[:],
                            ysb[:],
                            ACTF.Square,
                            accum_out=sqs[ot][:, idx : idx + 1],
                        )
                        nc.sync.dma_start(
                            y_dram[i, ot, :, nh * 512 : (nh + 1) * 512], ysb[:]
                        )

            # ---- stats reduce + AllReduce + per-channel affine -----------
            stat = sp.tile([128, 2 * OT], F32)
            for ot in range(OT):
                nc.vector.tensor_reduce(
                    stat[:, ot : ot + 1], sums[ot][:], axis=mybir.AxisListType.X,
                    op=ALU.add,
                )
                nc.vector.tensor_reduce(
                    stat[:, OT + ot : OT + ot + 1], sqs[ot][:],
                    axis=mybir.AxisListType.X, op=ALU.add,
                )
            ar_in = dr.tile([128, 2 * OT], F32)
            ar_out = dr.tile([128, 2 * OT], F32, addr_space="Shared")
            nc.sync.dma_start(ar_in[:], stat[:])
            if skip_collective:
                nc.gpsimd.dma_start(ar_out[:], ar_in[:])
            else:
                nc.gpsimd.collective_compute(
                    "AllReduce",
                    ALU.add,
                    replica_groups=[list(range(num_devices))],
                    ins=[ar_in.opt()],
                    outs=[ar_out.opt()],
                )
            ars = sp.tile([128, 2 * OT], F32)
            nc.sync.dma_start(ars[:], ar_out[:])

            meanv = sp.tile([128, OT], F32)
            nc.vector.tensor_scalar_mul(meanv[:], ars[:, 0:OT], 1.0 / COUNT)
            e2 = sp.tile([128, OT], F32)
            nc.vector.tensor_scalar(
                e2[:], ars[:, OT : 2 * OT], 1.0 / COUNT, EPS,
                op0=ALU.mult, op1=ALU.add,
            )
            m2 = sp.tile([128, OT], F32)
            nc.vector.scalar_tensor_tensor(
                m2[:], meanv[:], 0.0, meanv[:], op0=ALU.bypass, op1=ALU.mult
            )
            varep = sp.tile([128, OT], F32)
            nc.vector.scalar_tensor_tensor(
                varep[:], m2[:], -1.0, e2[:], op0=ALU.mult, op1=ALU.add
            )
            sq_ = sp.tile([128, OT], F32)
            nc.scalar.sqrt(sq_[:], varep[:])
            rstd = sp.tile([128, OT], F32)
            nc.vector.reciprocal(rstd[:], sq_[:])
            a_ = sp.tile([128, OT], F32)
            nc.vector.scalar_tensor_tensor(
                a_[:], g_sb[:], 0.0, rstd[:], op0=ALU.bypass, op1=ALU.mult
            )
            scalew = sp.tile([128, OT], F32)
            nc.vector.tensor_scalar_mul(scalew[:], a_[:], 0.5)
            ma = sp.tile([128, OT], F32)
            nc.vector.scalar_tensor_tensor(
                ma[:], meanv[:], 0.0, a_[:], op0=ALU.bypass, op1=ALU.mult
            )
            bhalf = sp.tile([128, OT], F32)
            nc.vector.tensor_scalar_mul(bhalf[:], b_sb[:], 0.5)
            biasw = sp.tile([128, OT], F32)
            nc.vector.scalar_tensor_tensor(
                biasw[:], ma[:], -0.5, bhalf[:], op0=ALU.mult, op1=ALU.add
            )

            # ---- pass B: normalize, LIF, permuted spike stripes ----------
            for j in range(B_SH if not skip_b else 0):
                stg = {}
                for t_ in range(T):
                    stg[t_] = sg.tile(
                        [128, NB, C], BF16, tag=f"stg{t_}", name=f"stg{j}_{t_}"
                    )
                for cb in range(KT):
                    w_tiles = []
                    for t_ in range(T):
                        i = t_ * B_SH + j
                        yt = bp.tile([128, N], F32, tag="yb")
                        nc.sync.dma_start(yt[:], y_dram[i, cb, :, :])
                        wt_ = bp.tile([128, N], F32, tag="wb")
                        nc.scalar.activation(
                            wt_[:],
                            yt[:],
                            ACTF.Identity,
                            bias=biasw[:, cb : cb + 1],
                            scale=scalew[:, cb : cb + 1],
                        )
                        w_tiles.append(wt_)
                    v_list = [w_tiles[0]]
                    cur = w_tiles[0]
                    for t_ in range(1, T):
                        reset_eng = (
                            nc.gpsimd
                            if (cb * 3 + t_) % 997 < int(lif_pool_frac * 997)
                            else nc.vector
                        )
                        vr = vp.tile([128, N], F32, tag="v")
                        reset_eng.scalar_tensor_tensor(
                            vr[:], cur[:], 1.0, cur[:], op0=ALU.is_lt, op1=ALU.mult
                        )
                        vn = vp.tile([128, N], F32, tag="v")
                        nc.vector.scalar_tensor_tensor(
                            vn[:], vr[:], 0.5, w_tiles[t_][:],
                            op0=ALU.mult, op1=ALU.add,
                        )
                        v_list.append(vn)
                        cur = vn
                    for t_ in range(T):
                        vv = v_list[t_][:].rearrange("p (n3 n2) -> p n2 n3", n2=4)
                        for n2 in range(4):
                            code = 6 * n2 + cb
                            pt, r = code % 8, code // 8
                            dst = stg[t_][:, pt, :].rearrange(
                                "p (n3 r) -> p r n3", r=3
                            )[:, r, :]
                            src = vv[:, n2, :]
                            sidx = ((j * KT + cb) * T + t_) * 4 + n2
                            eng = (
                                nc.vector
                                if sidx % 16 < stripe_dve_of16
                                else nc.gpsimd
                            )
                            eng.tensor_scalar(
                                dst, src, 1.0, None, op0=ALU.is_ge
                            )
                for t_ in range(T):
                    i = t_ * B_SH + j
                    ov = out.ap()[i].rearrange("(g pt p) c -> g p pt c", g=2, p=128)
                    for g in range(2):
                        nc.gpsimd.dma_start(
                            ov[g], stg[t_][:, g * 4 : (g + 1) * 4, :]
                        )

    nc.compile()
    return nc


def _bf16_split(a):
    """Round-to-nearest-even bf16 hi/lo split via uint32 ops (fast, no
    ml_dtypes elementwise casts)."""
    import ml_dtypes

    def to_bf16(f):
        u = f.view(np.uint32)
        rounded = (u + 0x7FFF + ((u >> 16) & 1)) >> 16
        return rounded.astype(np.uint16).view(ml_dtypes.bfloat16)

    hi = to_bf16(a)
    lo = to_bf16(a - hi.astype(np.float32))
    return hi, lo


def _prep_inputs(x, W, gamma, beta, host_prep):
    wt = np.ascontiguousarray(W.T.astype(np.float32))
    wth, wtl = _bf16_split(wt)
    idm = np.eye(128, dtype=np.float32)
    xv = np.asarray(x, dtype=np.float32).reshape(T, B, N, C)
    in_maps = []
    for s in range(N_CORES):
        common = {
            "wth": np.asarray(wth),
            "wtl": np.asarray(wtl),
            "gm": np.asarray(gamma, dtype=np.float32),
            "bt": np.asarray(beta, dtype=np.float32),
        }
        if host_prep:
            xsp = np.ascontiguousarray(
                xv[:, s * B_SH : (s + 1) * B_SH]
                .transpose(0, 1, 3, 2)
                .reshape(SL, C, N)
            )
            hi, lo = _bf16_split(xsp)
            common.update({"xth": np.asarray(hi), "xtl": np.asarray(lo)})
        else:
            common.update(
                {
                    "xs": np.ascontiguousarray(
                        xv[:, s * B_SH : (s + 1) * B_SH].reshape(SL, N, C)
                    ),
                    "idm": idm,
                }
            )
        in_maps.append(common)
    return in_maps


HOST_PREP = os.environ.get("KERNEL_HOST_PREP", "0") == "1"


def kernel(x, W, gamma, beta, T=4, **_unused):
    assert int(T) == 4
    if "nc" not in _CACHE:
        _CACHE["nc"] = build(host_prep=HOST_PREP)
    nc = _CACHE["nc"]
    in_maps = _prep_inputs(x, W, gamma, beta, HOST_PREP)
    res = bass_utils.run_bass_kernel_spmd(
        nc, in_maps, core_ids=list(range(N_CORES))
    )
    out = np.empty((T * B // 4 * 4, N, C), dtype=np.float32)  # (64, N, C)
    ov = out.reshape(4, B, N, C)
    for s in range(N_CORES):
        ov[:, s * B_SH : (s + 1) * B_SH] = res.results[s]["out"].reshape(
            4, B_SH, N, C
        )
    return out


if __name__ == "__main__":
    if os.environ.get("TLSIM"):
        from concourse.timeline_sim import TimelineSim

        nc = build(num_devices=1, skip_collective=True)
        ts = TimelineSim(nc, trace=False)
        t = ts.simulate()
        print(f"timeline_sim total = {t / 1e3:.1f} us")
    else:
        rng = np.random.default_rng(0)
        x = rng.standard_normal((T * B, N, C), dtype=np.float32)
        W = (rng.standard_normal((C, C)) / np.sqrt(C)).astype(np.float32)
        gamma = np.ones(C, np.float32)
        beta = np.zeros(C, np.float32)
        o = kernel(x, W, gamma, beta, 4)
        print("ran, out shape", o.shape, "mean spike", o.mean())
